# revision 10
# baseline (speedup 1.0000x reference)
"""Single-head causal attention (B=4, T=4096, n_embd=1024, head=64) on 8 trn2 cores.

Key-split scheme: core c -> batch b=c//2, half h=c%2.  Every core computes
ALL T queries of its batch but only HALF the keys: within each 512-wide
t-panel p, core h owns the contiguous 256 keys [512p+256h, 512p+256h+256),
i.e. global k-blocks {4p+2h, 4p+2h+1}.  Exact causal coverage (zero padding)
with an IDENTICAL instruction stream on every core; all per-core variation
lives in the data: a host-side roll of xt makes the owned keys sit at columns
[0:256) of every panel, and the mask table dtab[i,c] = ((c+256h) mod 512) - i
with two constant thresholds 128*(2h+{0,1}) drives the diagonal masking.

Math per panel pair (bf16 matmuls, fp32 psum):
  S^T[tk,tq] = K_blk^T.T @ Q^T          (panel-pair common k-blocks share one
                                         stationary load; psum [128,1024])
  P^T = exp(S^T / 8) -> bf16            (ScalarE, one op per k-block pair; no
                                         max-subtraction needed: S/8 ~ N(0,1))
  P^T *= (dtab >= thr)  on diagonal blocks only (VectorE)
  O_aug^T[65,1024] += V_aug_blk.T @ P^T (V_aug col 64 = ones -> row 64 of
                                         O_aug accumulates the denominator)

Perf structure (measured on HW):
 - matmuls pay ~170-280ns fixed latency + ~100ns LDWEIGHTS each -> panel
   pairs halve stationary loads; Q-projection stationary is zero-padded to
   [128,128] (64-wide stationaries are ~175ns slower per matmul).
 - software pipelining: S issues TWO k-block groups ahead of the exp-gated
   AV (3-deep psum ring), so the in-order PE queue never stalls on ScalarE.
 - s/proj psum tiles share one 3-deep pool ring (6 banks) + O_aug (2 banks);
   xt prefetch 3-deep, pt 4-deep to absorb engine jitter.
 - xt is host-packed [128, pair, nt, 2*512] so each pair DMA is
   16KB-contiguous per partition, in 4 chunks so projections start early.
 - output O_aug^T [65, T] is written in bf16, split across 3 DMA queues.
Host combine (unsharding): add the two per-pair partials, divide by the
summed denominator row, un-roll core h=1's panels, transpose.
"""

import numpy as np

B, T, NE, HD = 4, 4096, 1024, 64
QB = 512            # q-panel width
KB = 128            # k-block width
NP = T // QB        # 8 panels
NPR = NP // 2       # 4 panel pairs
NT = NE // 128      # 8 contraction tiles
LT = T // 2         # local key count per core

_CACHE = {}


def _build_program():
    import concourse.bass as bass
    import concourse.mybir as mybir
    import concourse.tile as tile

    f32 = mybir.dt.float32
    bf16 = mybir.dt.bfloat16
    AF = mybir.ActivationFunctionType
    MS = bass.MemorySpace
    nc = bass.Bass("TRN2", target_bir_lowering=True, debug=False,
                   enable_asserts=False)

    # xt packed [128, pair, nt, 2*QB] (pair-major, 16KB contiguous per line)
    xt_d = nc.dram_tensor("xt", [128, NPR, NT, 2 * QB], bf16,
                          kind="ExternalInput").ap()
    wkv_d = nc.dram_tensor("wkv", [NE, 128], bf16, kind="ExternalInput").ap()
    wqp_d = nc.dram_tensor("wqp", [NE, 128], bf16, kind="ExternalInput").ap()
    identh_d = nc.dram_tensor("identh", [128, 64], bf16, kind="ExternalInput").ap()
    dtab_d = nc.dram_tensor("dtab", [128, QB], f32, kind="ExternalInput").ap()
    thr_d = nc.dram_tensor("thr", [128, 2], f32, kind="ExternalInput").ap()
    out_d = nc.dram_tensor("out", [65, T], bf16, kind="ExternalOutput").ap()

    with tile.TileContext(nc) as tc:
        with (
            tc.tile_pool(name="consts", bufs=1) as cpool,
            tc.tile_pool(name="big", bufs=1) as bigpool,
            tc.tile_pool(name="xt", bufs=3) as xtpool,
            tc.tile_pool(name="pt", bufs=6) as ptpool,
            tc.tile_pool(name="ob", bufs=3) as obpool,
            tc.tile_pool(name="sps", bufs=3, space=MS.PSUM) as spool,
            tc.tile_pool(name="ops", bufs=1, space=MS.PSUM) as opool,
        ):
            # ---- constants ----
            wkv_sb = cpool.tile([128, NT, 128], bf16)
            nc.sync.dma_start(wkv_sb[:], wkv_d.rearrange("(nt p) m -> p nt m", p=128))
            wq_sb = cpool.tile([128, NT, 128], bf16)
            nc.sync.dma_start(wq_sb[:], wqp_d.rearrange("(nt p) m -> p nt m", p=128))
            identh = cpool.tile([128, 64], bf16)
            nc.sync.dma_start(identh[:], identh_d[:])
            dtab = cpool.tile([128, QB], f32)
            nc.sync.dma_start(dtab[:], dtab_d[:])
            thr = cpool.tile([128, 2], f32)
            nc.sync.dma_start(thr[:], thr_d[:])

            # ---- persistent sbuf state ----
            kvt = bigpool.tile([128, LT], bf16)        # 0:64 K^T, 64:128 V^T
            qt = bigpool.tile([64, T], bf16)           # rolled Q^T, all panels
            v_aug = bigpool.tile([128, 16 * 65], bf16)  # V natural + ones col
            nc.vector.memset(v_aug[:], 1.0)

            for m in range(NPR):
                # ---- projections for panel pair (2m, 2m+1) ----
                xt_sbs = []
                for k in range(4):
                    xt_sb = xtpool.tile([128, 2, 2, QB], bf16, tag=f"xt{k}")
                    nc.gpsimd.dma_start(
                        xt_sb[:],
                        xt_d[:, m, 2 * k:2 * k + 2].rearrange(
                            "p nt (two t) -> p nt two t", two=2))
                    xt_sbs.append(xt_sb)
                # KV projection: own 256 cols of both panels in one matmul
                kv_ps = spool.tile([128, 2, 256], f32, tag="sps")
                for ni in range(NT):
                    nc.tensor.matmul(kv_ps[:], wkv_sb[:, ni, :],
                                     xt_sbs[ni // 2][:, ni % 2, :, 0:256],
                                     start=(ni == 0), stop=(ni == NT - 1))
                nc.vector.tensor_copy(
                    kvt[:, m * 512:(m + 1) * 512], kv_ps[:])
                # Q projection per panel (padded stationary -> full col width)
                for pan in range(2):
                    q_ps = spool.tile([128, QB], f32, tag="sps")
                    for ni in range(NT):
                        nc.tensor.matmul(q_ps[:], wq_sb[:, ni, :],
                                         xt_sbs[ni // 2][:, ni % 2, pan, :],
                                         start=(ni == 0), stop=(ni == NT - 1))
                    nc.vector.tensor_copy(
                        qt[:, (2 * m + pan) * QB:(2 * m + pan + 1) * QB],
                        q_ps[0:64, :])
                # V natural for the 4 own k-blocks of this pair
                for j in range(4):
                    lkb = 4 * m + j
                    vt_ps = spool.tile([128, 2 * QB], bf16, tag="sps")
                    nc.tensor.transpose(
                        vt_ps[:, 0:64], kvt[64:128, lkb * KB:(lkb + 1) * KB],
                        identh[64:128, 0:64])
                    nc.vector.tensor_copy(v_aug[:, lkb * 65:lkb * 65 + 64],
                                          vt_ps[:, 0:64])

                # ---- attention for panels (2m, 2m+1), software-pipelined:
                # S for group g+1 issues before the exp-gated AV of group g so
                # the in-order PE queue never stalls on ScalarE.
                o_ps = opool.tile([65, 2 * QB], f32, tag="ops")
                ncom = 4 * m + 2
                ngrp = ncom + 1   # last group = the two extra k-blocks

                def emit_s(g):
                    s_ps = spool.tile([128, 2 * QB], f32, tag="sps")
                    if g < ncom:
                        for pan in range(2):
                            nc.tensor.matmul(
                                s_ps[:, pan * QB:(pan + 1) * QB],
                                kvt[0:64, g * KB:(g + 1) * KB],
                                qt[:, (2 * m + pan) * QB:(2 * m + pan + 1) * QB],
                                start=True, stop=True)
                    else:
                        for d in range(2):
                            nc.tensor.matmul(
                                s_ps[:, d * QB:(d + 1) * QB],
                                kvt[0:64, (ncom + d) * KB:(ncom + d + 1) * KB],
                                qt[:, (2 * m + 1) * QB:(2 * m + 2) * QB],
                                start=True, stop=True)
                    return s_ps

                s_cur = emit_s(0)
                s_next = emit_s(1) if ngrp > 1 else None
                for g in range(ngrp):
                    s_next2 = emit_s(g + 2) if g + 2 < ngrp else None
                    pt = ptpool.tile([128, 2 * QB], bf16, tag="pt")
                    nc.scalar.activation(pt[:], s_cur[:], AF.Exp,
                                         scale=float(HD) ** -0.5)
                    if g < ncom:
                        if g >= 4 * m:
                            # diagonal k-block of panel 2m: mask its half only
                            nc.vector.scalar_tensor_tensor(
                                pt[:, 0:QB], dtab[:],
                                thr[:, g - 4 * m:g - 4 * m + 1],
                                pt[:, 0:QB],
                                mybir.AluOpType.is_ge, mybir.AluOpType.mult)
                        for pan in range(2):
                            nc.tensor.matmul(
                                o_ps[:, pan * QB:(pan + 1) * QB],
                                v_aug[:, g * 65:g * 65 + 65],
                                pt[:, pan * QB:(pan + 1) * QB],
                                start=(g == 0), stop=False,
                                skip_group_check=True)
                    else:
                        for d in range(2):
                            nc.vector.scalar_tensor_tensor(
                                pt[:, d * QB:(d + 1) * QB], dtab[:],
                                thr[:, d:d + 1],
                                pt[:, d * QB:(d + 1) * QB],
                                mybir.AluOpType.is_ge, mybir.AluOpType.mult)
                        nc.tensor.matmul(
                            o_ps[:, QB:2 * QB],
                            v_aug[:, ncom * 65:ncom * 65 + 65],
                            pt[:, 0:QB],
                            start=False, stop=False, skip_group_check=True)
                        nc.tensor.matmul(
                            o_ps[:, QB:2 * QB],
                            v_aug[:, (ncom + 1) * 65:(ncom + 1) * 65 + 65],
                            pt[:, QB:2 * QB],
                            start=False, stop=True, skip_group_check=True)
                    s_cur, s_next = s_next, s_next2
                ob = obpool.tile([65, 2 * QB], bf16, tag="ob")
                nc.vector.tensor_copy(ob[:], o_ps[:])
                cols = slice(2 * m * QB, (2 * m + 2) * QB)
                nc.sync.dma_start(out_d[0:22, cols], ob[0:22, :])
                nc.scalar.dma_start(out_d[22:44, cols], ob[22:44, :])
                nc.gpsimd.dma_start(out_d[44:65, cols], ob[44:65, :])

    _legalize_matmul_waits(nc)
    return nc


def _legalize_matmul_waits(nc):
    """walrus' LW template encodes at most one sync-wait; hoist extra waits
    from Matmult instructions onto a preceding PE NoOp (same queue, so
    ordering semantics are identical)."""
    import concourse.mybir as mybir

    for f in nc.m.functions:
        for bb in f.blocks:
            new_insts = []
            for inst in bb.instructions:
                si = inst.sync_info
                if (si is not None and si.on_wait and len(si.on_wait) >= 2):
                    for w in si.on_wait:
                        nop = mybir.InstNoOp(
                            name=nc.get_next_instruction_name(),
                            text_hint="wait_hoist", bass_nofuse=True)
                        nop.engine = inst.engine
                        nop.sync_info = mybir.SyncInfo(
                            on_wait=[w], on_update=[])
                        new_insts.append(nop)
                    inst.sync_info = mybir.SyncInfo(
                        on_wait=[], on_update=list(si.on_update or []))
                new_insts.append(inst)
            del bb.instructions[:]
            for i in new_insts:
                bb.instructions.append(i)


def _make_inputs(x, Wq, Wk, Wv):
    import ml_dtypes
    bf = ml_dtypes.bfloat16
    wkv = np.ascontiguousarray(np.concatenate([Wk, Wv], axis=1).astype(bf))
    wqp = np.zeros((NE, 128), dtype=bf)
    wqp[:, 0:HD] = np.asarray(Wq).astype(bf)
    identh = np.zeros((128, 64), dtype=bf)
    identh[64:128, :] = np.eye(64, dtype=np.float32).astype(bf)

    col = np.arange(QB, dtype=np.int64)[None, :]
    row = np.arange(128, dtype=np.int64)[:, None]

    in_maps = []
    for c in range(8):
        b, h = c // 2, c % 2
        qp = (col + 256 * h) % 512               # rolled q index within panel
        dtab = np.ascontiguousarray((qp - row).astype(np.float32))
        thr = np.zeros((128, 2), dtype=np.float32)
        thr[:, 0] = 128.0 * (2 * h)
        thr[:, 1] = 128.0 * (2 * h + 1)
        xt = np.asarray(x[b]).T.astype(bf)       # [NE, T]
        xtr = xt.reshape(NE, NP, QB)
        xtr = np.roll(xtr, -256 * h, axis=2)     # per-core panel roll
        # pack [NE, NP, QB] -> [128 part, pair, nt, 2*QB]
        xp = xtr.reshape(NT, 128, NPR, 2 * QB).transpose(1, 2, 0, 3)
        in_maps.append({
            "xt": np.ascontiguousarray(xp),
            "wkv": wkv, "wqp": wqp, "identh": identh,
            "dtab": dtab, "thr": thr,
        })
    return in_maps


def kernel(x, Wq, Wk, Wv, _want_results=False, _trace=False):
    from concourse import bass_utils

    if "prog" not in _CACHE:
        _CACHE["prog"] = _build_program()
    nc = _CACHE["prog"]
    in_maps = _make_inputs(x, Wq, Wk, Wv)
    res = bass_utils.run_bass_kernel_spmd(nc, in_maps, core_ids=list(range(8)),
                                          trace=_trace)
    out = np.zeros((B, T, HD), dtype=np.float32)
    for b in range(B):
        o0 = np.asarray(res.results[2 * b]["out"], dtype=np.float32)
        o1 = np.asarray(res.results[2 * b + 1]["out"], dtype=np.float32)
        o1 = np.roll(o1.reshape(65, NP, QB), -256, axis=2).reshape(65, T)
        numer = o0[:HD] + o1[:HD]
        denom = o0[HD] + o1[HD]
        out[b] = (numer / denom).T
    if _want_results:
        return out, res
    return out
